# revision 19
# baseline (speedup 1.0000x reference)
"""GAT layer (N=8192, D=64) as a Bass/Tile kernel on 8 TRN2 NeuronCores.

Math (reference):
    h  = x @ W.T + b
    s1 = h @ a1 ; s2 = h @ a2                    # [N] each
    score[i,j] = s2[i] + s1[j]
    att = softmax_j(leaky_relu(score))
    out = att @ x

Reformulation used here:
    Fold the linear layer:  v = W.T @ [a1|a2], c_k = b.a_k
      p1 = x @ v1 ; p2 = x @ v2 ; s1 = p1 + c1 ; s2 = p2 + c2
    Softmax rows are shift invariant, so subtract p2[i] from row i:
      exp(lr(score) - p2[i]) = max( exp(sh1[j]),
                                    exp(0.01*sh1[j]) * exp(-0.99*p2[i]) )
      with sh1[j] = p1[j] + c1 + c2   (lr = leaky-relu, exp is monotone
      so exp(max(a,b)) = max(exp a, exp b))
    So with per-j-row scalars E1 = exp(sh1), F1 = exp(0.01*sh1) and a
    broadcast tile G2b[j,i] = exp(-0.99*p2[i]), the unnormalized weight
    tile (layout [j partitions, i free]) is ONE tensor_scalar op:
      e[j,i] = max( G2b[j,i] * F1[j],  E1[j] )
    The final matmul (with a ones-column appended to x to get the
    softmax denominator for free) accumulates over j in PSUM:
      outT[0:64, i] += x_ext[j,:].T @ e[j, i] ; Z[i] = outT[64, i]

Sharding: each core owns N/8 = 1024 query rows i (full x is only 2MB and
is replicated to every core), no collectives needed. Inputs are shipped
pre-permuted to partition-major layout (p, t, d) so every DMA is a flat
contiguous 2D transfer, spread over several engine DMA queues.
"""

import sys
import types

import ml_dtypes
import numpy as np

import concourse.bacc as bacc
import concourse.bass as bass
import concourse.mybir as mybir
import concourse.tile as tile
from concourse.bass_utils import run_bass_kernel_spmd


def _install_ntff_hook_shim():
    """The agent image's ``antenv`` lacks ``axon_hooks``; provide it so
    ``run_bass_kernel_spmd(trace=True)`` can capture NTFF profiles. The
    actual hook implementation ships with the axon boot package."""
    if "antenv.axon_hooks" in sys.modules:
        return
    try:
        from trn_agent_boot.trn_boot import _ntff_profile_via_ctypes

        hook = _ntff_profile_via_ctypes("/opt/axon/libaxon_pjrt.so")
        mod = types.ModuleType("antenv.axon_hooks")
        mod._hook = hook
        mod.get_axon_ntff_profile_hook = lambda: mod._hook
        mod.set_axon_ntff_profile_hook = lambda h: setattr(mod, "_hook", h)
        sys.modules["antenv.axon_hooks"] = mod
    except Exception:
        pass


_install_ntff_hook_shim()

N, D = 8192, 64
NCORES = 8
RB = N // NCORES          # rows (i) per core = 1024
NT = N // 128             # j tiles of 128 = 64
BT = RB // 128            # i tiles per core = 8
F32 = mybir.dt.float32
BF16 = mybir.dt.bfloat16
EXP = mybir.ActivationFunctionType.Exp
ADD = mybir.AluOpType.add
MUL = mybir.AluOpType.mult
MAX = mybir.AluOpType.max
AX_X = mybir.AxisListType.X


def build_bass() -> bass.Bass:
    nc = bacc.Bacc(None)
    # partition-major (p, t, d) layouts, prepared on the host
    xp_d = nc.declare_dram_parameter("xp", [128, NT * D], F32, isOutput=False)
    xbf_d = nc.declare_dram_parameter(
        "xbf", [128, NT * (D + 1)], BF16, isOutput=False
    )
    xbk_d = nc.declare_dram_parameter("xblk", [128, BT * D], F32, isOutput=False)
    W_d = nc.declare_dram_parameter("W", [D, D], F32, isOutput=False)
    b_d = nc.declare_dram_parameter("b", [D, 1], F32, isOutput=False)
    a_d = nc.declare_dram_parameter("a", [2 * D, 1], F32, isOutput=False)
    id_d = nc.declare_dram_parameter("ident", [128, 128], F32, isOutput=False)
    out_d = nc.declare_dram_parameter("out", [128, BT * D], F32, isOutput=True)

    with tile.TileContext(nc) as tc:
        with (
            tc.tile_pool(name="persist", bufs=1) as persist,
            tc.tile_pool(name="small", bufs=1) as small,
            tc.tile_pool(name="work", bufs=2) as work,
            tc.tile_pool(name="epool", bufs=3) as epool,
            tc.tile_pool(name="opool", bufs=2) as opool,
            tc.tile_pool(name="psumA", bufs=3, space="PSUM") as psumA,
            tc.tile_pool(name="psumB", bufs=1, space="PSUM") as psumB,
        ):
            # ------- small loads (vector HW queue) -------
            xblk_f = small.tile([128, BT * D], F32)
            nc.sync.dma_start(xblk_f, xbk_d[:, :])
            xblk_sb = xblk_f.rearrange("p (t d) -> p t d", t=BT)
            W_sb = small.tile([D, D], F32)
            nc.sync.dma_start(W_sb, W_d[:, :])
            b_sb = small.tile([D, 1], F32)
            nc.sync.dma_start(b_sb, b_d[:, :])
            a_sb = small.tile([D, 2], F32)
            nc.sync.dma_start(
                a_sb,
                bass.AP(
                    tensor=a_d[:, :].tensor,
                    offset=a_d[:, :].offset,
                    ap=[[1, D], [D, 2]],
                ),
            )
            ident = small.tile([128, 128], F32)
            nc.sync.dma_start(ident, id_d[:, :])
            ones_row = small.tile([1, 128], F32)
            nc.vector.memset(ones_row, 1.0)
            ones_bf = small.tile([1, 128], BF16)
            nc.vector.memset(ones_bf, 1.0)

            # ------- x loads: flat contiguous 2D chunks -------
            # f32 x on sync + tensor queues (4 chunks each)
            x_flat = persist.tile([128, NT * D], F32)
            x_sb = x_flat.rearrange("p (t d) -> p t d", t=NT)
            CW = 8 * D  # 8 j-tiles worth of columns
            for c in range(8):
                eng = nc.sync if c < 4 else nc.gpsimd
                eng.dma_start(
                    x_flat[:, c * CW : (c + 1) * CW],
                    xp_d[:, c * CW : (c + 1) * CW],
                )
            # bf16 x (with ones column folded in) on scalar queue
            xbf_flat = persist.tile([128, NT * (D + 1)], BF16)
            x_bf = xbf_flat.rearrange("p (t d) -> p t d", t=NT)
            CWB = 8 * (D + 1)
            for c in range(8):
                nc.scalar.dma_start(
                    xbf_flat[:, c * CWB : (c + 1) * CWB],
                    xbf_d[:, c * CWB : (c + 1) * CWB],
                )

            # ---------------- tiny projections on PE ----------------
            # v = W.T @ [a1|a2]  [64,2] ;  c = [b.a1, b.a2]  [1,2]
            v_ps = psumA.tile([D, 2], F32, tag="ps", name="v_ps")
            nc.tensor.matmul(v_ps, lhsT=W_sb, rhs=a_sb, start=True, stop=True)
            v_sb = small.tile([D, 2], F32)
            nc.vector.tensor_copy(out=v_sb, in_=v_ps)

            c_ps = psumA.tile([1, 2], F32, tag="ps", name="c_ps")
            nc.tensor.matmul(c_ps, lhsT=b_sb, rhs=a_sb, start=True, stop=True)
            c_sb = small.tile([1, 2], F32)
            nc.vector.tensor_copy(out=c_sb, in_=c_ps)

            # c12 = (c1 + c2) broadcast down 128 partitions
            cb_ps = psumA.tile([128, 2], F32, tag="ps", name="cb_ps")
            nc.tensor.matmul(cb_ps, lhsT=ones_row, rhs=c_sb, start=True, stop=True)
            c12 = small.tile([128, 1], F32)
            nc.vector.tensor_reduce(out=c12, in_=cb_ps, axis=AX_X, op=ADD)
            c12s = small.tile([128, 1], F32)
            nc.vector.tensor_scalar(
                out=c12s, in0=c12, scalar1=0.01, scalar2=None, op0=MUL
            )

            # v1 / v2 rows (via PE transpose) and partition broadcasts
            v1r_ps = psumA.tile([1, D], F32, tag="ps", name="v1r_ps")
            nc.tensor.transpose(v1r_ps, v_sb[:, 0:1], ident[:D, :D])
            v1row = small.tile([1, D], F32)
            nc.vector.tensor_copy(out=v1row, in_=v1r_ps)
            v2r_ps = psumA.tile([1, D], F32, tag="ps", name="v2r_ps")
            nc.tensor.transpose(v2r_ps, v_sb[:, 1:2], ident[:D, :D])
            v2row = small.tile([1, D], F32)
            nc.vector.tensor_copy(out=v2row, in_=v2r_ps)

            v1b_ps = psumA.tile([128, D], F32, tag="ps", name="v1b_ps")
            nc.tensor.matmul(
                v1b_ps, lhsT=ones_row, rhs=v1row, start=True, stop=True
            )
            v1b = small.tile([128, D], F32)
            nc.vector.tensor_copy(out=v1b, in_=v1b_ps)
            v2b_ps = psumA.tile([128, D], F32, tag="ps", name="v2b_ps")
            nc.tensor.matmul(
                v2b_ps, lhsT=ones_row, rhs=v2row, start=True, stop=True
            )
            v2b = small.tile([128, D], F32)
            nc.vector.tensor_copy(out=v2b, in_=v2b_ps)

            # ---------------- p2 for this block -> G2b ----------------
            # p2cols[p, t] = x_blk[t*128+p, :] @ v2   (DVE mult + reduce)
            v2b_b = bass.AP(
                tensor=v2b.tensor,
                offset=v2b.offset,
                ap=[v2b.ap[0], [0, BT], v2b.ap[1]],
            )
            tmp2 = work.tile([128, BT, D], F32, tag="tmp2", name="tmp2")
            nc.vector.tensor_mul(tmp2, xblk_sb, v2b_b)
            p2cols = small.tile([128, BT], F32)
            nc.vector.tensor_reduce(out=p2cols, in_=tmp2, axis=AX_X, op=ADD)
            # flatten p2cols into a [1, 1024] row via 8 single-col transposes,
            # exp(-0.99 * .) into bf16, then broadcast down 128 partitions
            G2b = persist.tile([128, RB], BF16)
            for h in range(2):
                p2r_ps = psumA.tile([1, 512], F32, tag="ps", name="p2r_ps")
                for t4 in range(4):
                    t = h * 4 + t4
                    nc.tensor.transpose(
                        p2r_ps[:, t4 * 128 : (t4 + 1) * 128],
                        p2cols[:, t : t + 1],
                        ident,
                    )
                g2row = small.tile([1, 512], BF16, tag="g2row", name="g2row")
                nc.scalar.activation(out=g2row, in_=p2r_ps, func=EXP, scale=-0.99)
                gb_ps = psumA.tile([128, 512], F32, tag="ps", name="gb_ps")
                nc.tensor.matmul(
                    gb_ps, lhsT=ones_bf, rhs=g2row, start=True, stop=True
                )
                nc.vector.tensor_copy(
                    out=G2b[:, h * 512 : (h + 1) * 512], in_=gb_ps
                )

            # ---------------- s1 columns + exps ----------------
            # s1c[p, jt] = sum_d x[jt*128+p, d] * v1[d]
            s1c = small.tile([128, NT], F32)
            E1c = small.tile([128, NT], F32)
            F1c = small.tile([128, NT], F32)
            v1b_b = bass.AP(
                tensor=v1b.tensor,
                offset=v1b.offset,
                ap=[v1b.ap[0], [0, 8], v1b.ap[1]],
            )
            for c in range(8):
                tmp = work.tile([128, 8, D], F32, tag="tmp", name="tmp")
                nc.vector.tensor_mul(
                    tmp, x_sb[:, 8 * c : 8 * (c + 1), :], v1b_b
                )
                nc.vector.tensor_reduce(
                    out=s1c[:, 8 * c : 8 * (c + 1)], in_=tmp, axis=AX_X, op=ADD
                )
            for c in range(4):
                nc.scalar.activation(
                    out=E1c[:, 16 * c : 16 * (c + 1)],
                    in_=s1c[:, 16 * c : 16 * (c + 1)],
                    func=EXP,
                    bias=c12,
                    scale=1.0,
                )
                nc.scalar.activation(
                    out=F1c[:, 16 * c : 16 * (c + 1)],
                    in_=s1c[:, 16 * c : 16 * (c + 1)],
                    func=EXP,
                    bias=c12s,
                    scale=0.01,
                )

            # ---------------- main loop over j tiles ----------------
            acc0 = psumB.tile([D + 1, 512], F32, tag="acc0", name="acc0")
            acc1 = psumB.tile([D + 1, 512], F32, tag="acc1", name="acc1")
            accs = [acc0, acc1]
            for jt in range(NT):
                e_t = epool.tile([128, RB], BF16, tag="e", name="e_t")
                # e[j,i] = max(G2b[j,i] * F1[j], E1[j])
                nc.vector.tensor_scalar(
                    out=e_t,
                    in0=G2b,
                    scalar1=F1c[:, jt : jt + 1],
                    scalar2=E1c[:, jt : jt + 1],
                    op0=MUL,
                    op1=MAX,
                )
                for h in range(2):
                    nc.tensor.matmul(
                        accs[h],
                        lhsT=x_bf[:, jt, :],
                        rhs=e_t[:, h * 512 : (h + 1) * 512],
                        start=(jt == 0),
                        stop=(jt == NT - 1),
                    )

            # ---------------- epilogue: normalize + store ----------------
            outT = small.tile([D + 1, RB], F32)
            for h in range(2):
                nc.vector.tensor_copy(
                    out=outT[:, h * 512 : (h + 1) * 512], in_=accs[h]
                )
            out_flat = small.tile([128, BT * D], F32)
            out_sb = out_flat.rearrange("p (t d) -> p t d", t=BT)
            for t in range(BT):
                tp2 = psumA.tile([128, D + 1], F32, tag="ps", name="tp2")
                nc.tensor.transpose(
                    tp2, outT[:, t * 128 : (t + 1) * 128], ident[: D + 1, : D + 1]
                )
                rcol = opool.tile([128, 1], F32, tag="rcol", name="rcol")
                nc.vector.reciprocal(rcol, tp2[:, D : D + 1])
                nc.vector.tensor_scalar(
                    out=out_sb[:, t, :],
                    in0=tp2[:, 0:D],
                    scalar1=rcol,
                    scalar2=None,
                    op0=MUL,
                )
            nc.scalar.dma_start(out_d[:, :], out_flat)

    nc.finalize()
    return nc


def _execute(inputs: dict, trace: bool = False):
    x = np.ascontiguousarray(np.asarray(inputs["x"], dtype=np.float32))
    W = np.ascontiguousarray(np.asarray(inputs["W"], dtype=np.float32))
    b = np.ascontiguousarray(
        np.asarray(inputs["b"], dtype=np.float32).reshape(D, 1)
    )
    a = np.ascontiguousarray(
        np.asarray(inputs["a"], dtype=np.float32).reshape(2 * D, 1)
    )
    assert x.shape == (N, D) and W.shape == (D, D)

    # partition-major permutations: (t*128+p, d) -> (p, t*D+d)
    xp = np.ascontiguousarray(
        x.reshape(NT, 128, D).transpose(1, 0, 2).reshape(128, NT * D)
    )
    xe = np.concatenate([x, np.ones((N, 1), np.float32)], axis=1)
    xbf = np.ascontiguousarray(
        xe.reshape(NT, 128, D + 1)
        .transpose(1, 0, 2)
        .reshape(128, NT * (D + 1))
        .astype(ml_dtypes.bfloat16)
    )
    ident = np.eye(128, dtype=np.float32)

    nc = build_bass()
    in_maps = []
    for c in range(NCORES):
        xblk = x[c * RB : (c + 1) * RB]
        xbk = np.ascontiguousarray(
            xblk.reshape(BT, 128, D).transpose(1, 0, 2).reshape(128, BT * D)
        )
        in_maps.append(
            {
                "xp": xp,
                "xbf": xbf,
                "xblk": xbk,
                "W": W,
                "b": b,
                "a": a,
                "ident": ident,
            }
        )
    res = run_bass_kernel_spmd(
        nc, in_maps, core_ids=list(range(NCORES)), trace=trace
    )
    # un-permute each core's output: (p, t*D+d) -> (t*128+p, d)
    outs = []
    for r in res.results:
        o = r["out"].reshape(128, BT, D).transpose(1, 0, 2).reshape(RB, D)
        outs.append(o)
    out = np.ascontiguousarray(np.concatenate(outs, axis=0))
    return out, res


def kernel(x, W, b, a):
    out, _ = _execute({"x": x, "W": W, "b": b, "a": a})
    return out


# revision 29
# speedup vs baseline: 1.1744x; 1.1744x over previous
"""GAT layer (N=8192, D=64) as a Bass/Tile kernel on 8 TRN2 NeuronCores.

Math (reference):
    h  = x @ W.T + b
    s1 = h @ a1 ; s2 = h @ a2                    # [N] each
    score[i,j] = s2[i] + s1[j]
    att = softmax_j(leaky_relu(score))
    out = att @ x

Reformulation used here:
    Fold the linear layer:  v = W.T @ [a1|a2], c_k = b.a_k
      p1 = x @ v1 ; p2 = x @ v2 ; s1 = p1 + c1 ; s2 = p2 + c2
    Softmax rows are shift invariant, so subtract p2[i] from row i:
      exp(lr(score) - p2[i]) = max( exp(sh1[j]),
                                    exp(0.01*sh1[j]) * exp(-0.99*p2[i]) )
      with sh1[j] = p1[j] + c1 + c2   (lr = leaky-relu, exp is monotone
      so exp(max(a,b)) = max(exp a, exp b))
    So with per-j-row scalars E1 = exp(sh1), F1 = exp(0.01*sh1) and a
    broadcast tile G2b[j,i] = exp(-0.99*p2[i]), the unnormalized weight
    tile (layout [j partitions, i free]) is ONE tensor_scalar op:
      e[j,i] = max( G2b[j,i] * F1[j],  E1[j] )
    The final matmul (with a ones-column appended to x to get the
    softmax denominator for free) accumulates over j in PSUM:
      outT[0:64, i] += x_ext[j,:].T @ e[j, i] ; Z[i] = outT[64, i]

Sharding: each core owns N/8 = 1024 query rows i (full x is only 2MB and
is replicated to every core), no collectives needed. Inputs are shipped
pre-permuted to partition-major layout (p, t, d) so every DMA is a flat
contiguous 2D transfer. Engine roles: DVE runs the 64 fused
mult+max tensor_scalar tiles (the critical stream, ~35us), PE runs the
128 accumulating matmuls (~32us, overlapped), ACT does exps and
PSUM->SBUF copies, gpsimd does the s1 elementwise multiplies.
"""

import sys
import types

import ml_dtypes
import numpy as np

import concourse.bacc as bacc
import concourse.bass as bass
import concourse.mybir as mybir
import concourse.tile as tile
from concourse.bass_utils import run_bass_kernel_spmd


def _install_ntff_hook_shim():
    """The agent image's ``antenv`` lacks ``axon_hooks``; provide it so
    ``run_bass_kernel_spmd(trace=True)`` can capture NTFF profiles. The
    actual hook implementation ships with the axon boot package."""
    if "antenv.axon_hooks" in sys.modules:
        return
    try:
        from trn_agent_boot.trn_boot import _ntff_profile_via_ctypes

        hook = _ntff_profile_via_ctypes("/opt/axon/libaxon_pjrt.so")
        mod = types.ModuleType("antenv.axon_hooks")
        mod._hook = hook
        mod.get_axon_ntff_profile_hook = lambda: mod._hook
        mod.set_axon_ntff_profile_hook = lambda h: setattr(mod, "_hook", h)
        sys.modules["antenv.axon_hooks"] = mod
    except Exception:
        pass


_install_ntff_hook_shim()

N, D = 8192, 64
NCORES = 8
RB = N // NCORES          # rows (i) per core = 1024
NT = N // 128             # j tiles of 128 = 64
BT = RB // 128            # i tiles per core = 8
F32 = mybir.dt.float32
BF16 = mybir.dt.bfloat16
EXP = mybir.ActivationFunctionType.Exp
ADD = mybir.AluOpType.add
MUL = mybir.AluOpType.mult
MAX = mybir.AluOpType.max
AX_X = mybir.AxisListType.X
PKW = D + 131 + BT * D  # packed small-input width


def build_bass() -> bass.Bass:
    nc = bacc.Bacc(None)
    # partition-major (p, t, d) layouts, prepared on the host
    xp_d = nc.declare_dram_parameter("xp", [128, NT * D], F32, isOutput=False)
    xbf_d = nc.declare_dram_parameter(
        "xbf", [128, NT * 128], BF16, isOutput=False
    )
    pk_d = nc.declare_dram_parameter("pack", [128, PKW], F32, isOutput=False)
    out_d = nc.declare_dram_parameter("out", [128, BT * D], F32, isOutput=True)

    with tile.TileContext(nc) as tc:
        with (
            tc.tile_pool(name="persist", bufs=1) as persist,
            tc.tile_pool(name="small", bufs=1) as small,
            tc.tile_pool(name="work", bufs=3) as work,
            tc.tile_pool(name="epool", bufs=6) as epool,
            tc.tile_pool(name="opool", bufs=2) as opool,
            tc.tile_pool(name="psumA", bufs=3, space="PSUM") as psumA,
            tc.tile_pool(name="psumB", bufs=1, space="PSUM") as psumB,
        ):
            # ------- all small inputs arrive in ONE packed DMA -------
            pk = small.tile([128, PKW], F32)
            nc.sync.dma_start(pk, pk_d[:, :])
            W_sb = pk[0:D, 0:D]
            b_sb = pk[0:D, D : D + 1]
            a_sb = pk[0:D, D + 1 : D + 3]
            ident = pk[:, D + 3 : D + 3 + 128]
            xblk_sb = pk[:, D + 131 : D + 131 + BT * D].rearrange(
                "p (t d) -> p t d", t=BT
            )
            ones_row = small.tile([1, 128], F32)
            nc.vector.memset(ones_row, 1.0)
            ones_bf = small.tile([1, 128], BF16)
            nc.vector.memset(ones_bf, 1.0)

            # ------- x loads: flat contiguous 2D chunks on the SP queue -------
            # (each dma_start costs ~2us of descriptor generation on the
            # issuing engine, so few, large, contiguous transfers win)
            x_flat = persist.tile([128, NT * D], F32)
            x_sb = x_flat.rearrange("p (t d) -> p t d", t=NT)
            CW = 16 * D  # 16 j-tiles worth of columns
            for c in range(4):
                nc.sync.dma_start(
                    x_flat[:, c * CW : (c + 1) * CW],
                    xp_d[:, c * CW : (c + 1) * CW],
                )
            # bf16 x (with ones column folded in) on scalar queue
            xbf_flat = persist.tile([128, NT * 128], BF16)
            x_bf = xbf_flat.rearrange("p (t d) -> p t d", t=NT)
            CWB = 32 * 128
            for c in range(2):
                nc.scalar.dma_start(
                    xbf_flat[:, c * CWB : (c + 1) * CWB],
                    xbf_d[:, c * CWB : (c + 1) * CWB],
                )

            # ---------------- tiny projections on PE ----------------
            # v = W.T @ [a1|a2]  [64,2] ;  c = [b.a1, b.a2]  [1,2]
            v_ps = psumA.tile([D, 2], F32, tag="ps", name="v_ps")
            nc.tensor.matmul(v_ps, lhsT=W_sb, rhs=a_sb, start=True, stop=True)
            v_sb = small.tile([D, 2], F32)
            nc.scalar.copy(out=v_sb, in_=v_ps)

            c_ps = psumA.tile([1, 2], F32, tag="ps", name="c_ps")
            nc.tensor.matmul(c_ps, lhsT=b_sb, rhs=a_sb, start=True, stop=True)
            c_sb = small.tile([1, 2], F32)
            nc.scalar.copy(out=c_sb, in_=c_ps)

            # c12 = (c1 + c2) broadcast down 128 partitions
            cb_ps = psumA.tile([128, 2], F32, tag="ps", name="cb_ps")
            nc.tensor.matmul(cb_ps, lhsT=ones_row, rhs=c_sb, start=True, stop=True)
            c12 = small.tile([128, 1], F32)
            nc.vector.tensor_reduce(out=c12, in_=cb_ps, axis=AX_X, op=ADD)
            c12s = small.tile([128, 1], F32)
            nc.vector.tensor_scalar(
                out=c12s, in0=c12, scalar1=0.01, scalar2=None, op0=MUL
            )

            # v1 / v2 rows (via PE transpose) and partition broadcasts
            v1r_ps = psumA.tile([1, D], F32, tag="ps", name="v1r_ps")
            nc.tensor.transpose(v1r_ps, v_sb[:, 0:1], ident[:D, :D])
            v1row = small.tile([1, D], F32)
            nc.scalar.copy(out=v1row, in_=v1r_ps)
            v2r_ps = psumA.tile([1, D], F32, tag="ps", name="v2r_ps")
            nc.tensor.transpose(v2r_ps, v_sb[:, 1:2], ident[:D, :D])
            v2row = small.tile([1, D], F32)
            nc.scalar.copy(out=v2row, in_=v2r_ps)

            v1b_ps = psumA.tile([128, D], F32, tag="ps", name="v1b_ps")
            nc.tensor.matmul(
                v1b_ps, lhsT=ones_row, rhs=v1row, start=True, stop=True
            )
            v1b = small.tile([128, D], F32)
            nc.scalar.copy(out=v1b, in_=v1b_ps)
            v2b_ps = psumA.tile([128, D], F32, tag="ps", name="v2b_ps")
            nc.tensor.matmul(
                v2b_ps, lhsT=ones_row, rhs=v2row, start=True, stop=True
            )
            v2b = small.tile([128, D], F32)
            nc.scalar.copy(out=v2b, in_=v2b_ps)

            # ---------------- p2 for this block -> G2b ----------------
            # p2cols[p, t] = x_blk[t*128+p, :] @ v2   (DVE mult + reduce)
            v2b_b = bass.AP(
                tensor=v2b.tensor,
                offset=v2b.offset,
                ap=[v2b.ap[0], [0, BT], v2b.ap[1]],
            )
            tmp2 = work.tile([128, BT, D], F32, tag="tmp2", name="tmp2")
            nc.gpsimd.tensor_mul(tmp2, xblk_sb, v2b_b)
            p2cols = small.tile([128, BT], F32)
            nc.vector.tensor_reduce(out=p2cols, in_=tmp2, axis=AX_X, op=ADD)
            # flatten p2cols into a [1, 1024] row via 8 single-col transposes,
            # exp(-0.99 * .) into bf16, then broadcast down 128 partitions
            G2b = persist.tile([128, RB], BF16)
            for h in range(2):
                p2r_ps = psumA.tile([1, 512], F32, tag="ps", name="p2r_ps")
                for t4 in range(4):
                    t = h * 4 + t4
                    nc.tensor.transpose(
                        p2r_ps[:, t4 * 128 : (t4 + 1) * 128],
                        p2cols[:, t : t + 1],
                        ident,
                    )
                g2row = small.tile([1, 512], BF16, tag="g2row", name="g2row")
                nc.scalar.activation(out=g2row, in_=p2r_ps, func=EXP, scale=-0.99)
                gb_ps = psumA.tile([128, 512], F32, tag="ps", name="gb_ps")
                nc.tensor.matmul(
                    gb_ps, lhsT=ones_bf, rhs=g2row, start=True, stop=True
                )
                nc.scalar.copy(
                    out=G2b[:, h * 512 : (h + 1) * 512], in_=gb_ps
                )

            # ---------------- s1 columns + exps ----------------
            # s1c[p, jt] = sum_d x[jt*128+p, d] * v1[d]
            s1c = small.tile([128, NT], F32)
            E1c = small.tile([128, NT], F32)
            F1c = small.tile([128, NT], F32)
            v1b_b = bass.AP(
                tensor=v1b.tensor,
                offset=v1b.offset,
                ap=[v1b.ap[0], [0, 8], v1b.ap[1]],
            )
            for c in range(8):
                tmp = work.tile([128, 8, D], F32, tag="tmp", name="tmp")
                nc.gpsimd.tensor_mul(
                    tmp, x_sb[:, 8 * c : 8 * (c + 1), :], v1b_b
                )
                nc.vector.tensor_reduce(
                    out=s1c[:, 8 * c : 8 * (c + 1)], in_=tmp, axis=AX_X, op=ADD
                )
            for c in range(8):
                nc.scalar.activation(
                    out=E1c[:, 8 * c : 8 * (c + 1)],
                    in_=s1c[:, 8 * c : 8 * (c + 1)],
                    func=EXP,
                    bias=c12,
                    scale=1.0,
                )
                nc.scalar.activation(
                    out=F1c[:, 8 * c : 8 * (c + 1)],
                    in_=s1c[:, 8 * c : 8 * (c + 1)],
                    func=EXP,
                    bias=c12s,
                    scale=0.01,
                )

            # ---------------- main loop over j tiles ----------------
            acc0 = psumB.tile([128, 512], F32, tag="acc0", name="acc0")
            acc1 = psumB.tile([128, 512], F32, tag="acc1", name="acc1")
            accs = [acc0, acc1]
            for jt in range(NT):
                e_t = epool.tile([128, RB], BF16, tag="e", name="e_t")
                # e[j,i] = max(G2b[j,i] * F1[j], E1[j])
                nc.vector.tensor_scalar(
                    out=e_t,
                    in0=G2b,
                    scalar1=F1c[:, jt : jt + 1],
                    scalar2=E1c[:, jt : jt + 1],
                    op0=MUL,
                    op1=MAX,
                )
                for h in range(2):
                    nc.tensor.matmul(
                        accs[h],
                        lhsT=x_bf[:, jt, 0:128],
                        rhs=e_t[:, h * 512 : (h + 1) * 512],
                        start=(jt == 0),
                        stop=(jt == NT - 1),
                    )

            # ---------------- epilogue: normalize + store ----------------
            outT = small.tile([D + 1, RB], F32)
            for h in range(2):
                nc.scalar.copy(
                    out=outT[:, h * 512 : (h + 1) * 512],
                    in_=accs[h][0 : D + 1, :],
                )
            out_flat = small.tile([128, BT * D], F32)
            out_sb = out_flat.rearrange("p (t d) -> p t d", t=BT)
            for t in range(BT):
                tp2 = psumA.tile([128, D + 1], F32, tag="ps", name="tp2")
                nc.tensor.transpose(
                    tp2, outT[:, t * 128 : (t + 1) * 128], ident[: D + 1, : D + 1]
                )
                rcol = opool.tile([128, 1], F32, tag="rcol", name="rcol")
                nc.vector.reciprocal(rcol, tp2[:, D : D + 1])
                nc.vector.tensor_scalar(
                    out=out_sb[:, t, :],
                    in0=tp2[:, 0:D],
                    scalar1=rcol,
                    scalar2=None,
                    op0=MUL,
                )
            nc.scalar.dma_start(out_d[:, :], out_flat)

    nc.finalize()
    return nc


def _execute(inputs: dict, trace: bool = False):
    x = np.ascontiguousarray(np.asarray(inputs["x"], dtype=np.float32))
    W = np.ascontiguousarray(np.asarray(inputs["W"], dtype=np.float32))
    b = np.ascontiguousarray(
        np.asarray(inputs["b"], dtype=np.float32).reshape(D, 1)
    )
    a = np.ascontiguousarray(
        np.asarray(inputs["a"], dtype=np.float32).reshape(2 * D, 1)
    )
    assert x.shape == (N, D) and W.shape == (D, D)

    # partition-major permutations: (t*128+p, d) -> (p, t*D+d)
    xp = np.ascontiguousarray(
        x.reshape(NT, 128, D).transpose(1, 0, 2).reshape(128, NT * D)
    )
    xe = np.concatenate(
        [x, np.ones((N, 1), np.float32), np.zeros((N, 127 - D), np.float32)],
        axis=1,
    )
    xbf = np.ascontiguousarray(
        xe.reshape(NT, 128, 128)
        .transpose(1, 0, 2)
        .reshape(128, NT * 128)
        .astype(ml_dtypes.bfloat16)
    )
    nc = build_bass()
    in_maps = []
    for c in range(NCORES):
        xblk = x[c * RB : (c + 1) * RB]
        xbk = xblk.reshape(BT, 128, D).transpose(1, 0, 2).reshape(128, BT * D)
        pack = np.zeros((128, PKW), np.float32)
        pack[0:D, 0:D] = W
        pack[0:D, D] = b[:, 0]
        pack[0:D, D + 1] = a[:D, 0]
        pack[0:D, D + 2] = a[D:, 0]
        pack[:, D + 3 : D + 131] = np.eye(128, dtype=np.float32)
        pack[:, D + 131 :] = xbk
        in_maps.append({"xp": xp, "xbf": xbf, "pack": pack})
    res = run_bass_kernel_spmd(
        nc, in_maps, core_ids=list(range(NCORES)), trace=trace
    )
    # un-permute each core's output: (p, t*D+d) -> (t*128+p, d)
    outs = []
    for r in res.results:
        o = r["out"].reshape(128, BT, D).transpose(1, 0, 2).reshape(RB, D)
        outs.append(o)
    out = np.ascontiguousarray(np.concatenate(outs, axis=0))
    return out, res


def kernel(x, W, b, a):
    out, _ = _execute({"x": x, "W": W, "b": b, "a": a})
    return out
